# revision 1
# baseline (speedup 1.0000x reference)
"""Trainium2 Bass kernel for a 2-hidden-layer LIF spiking network.

Math (per timestep t, per layer):
    v = 0.9*y + cur ;  spike s = (v >= 1) ;  y = v*(1-s) = v*u  with u = (v < 1)
Layer currents:
    cur1 = x_t @ W_ih            (x binary, precomputable for ALL t)
    cur2 = s1 @ W_hh = colsum(W_hh) - u1 @ W_hh
    cur3 = s2 @ W_ho = colsum(W_ho) - u2 @ W_ho
Output: rate = mean_t s_out = 1 - sum_t(u_out)/T

Key restructurings:
  * Layer 1's recurrence does not depend on layer 2, so all three matmuls are
    batched over the full (T*B) column space; only the cheap elementwise LIF
    scans are sequential in t.
  * Weights are split W = W_hi + W_lo (both fp16). The moving operands (x and
    the spike complements u) are exactly representable in fp16, so the pair of
    fp16 matmuls accumulated in fp32 PSUM reproduces the fp32 product to
    ~2^-18 relative — while streaming 2x faster than native fp32 (4 cyc/row)
    and enabling fast weight load.

Sharding: data-parallel over batch (256/8 = 32 rows per core), weights
replicated, no cross-core communication.

Per-core schedule:
  Phase A (W_ih resident): mm1 over blocks of 10 steps (N=320 matmuls),
    LIF1 scan fused per block, spill u1 (fp16 {0,1}) to DRAM scratch.
  Phase B (W_hh resident): stream u1 back, mm2 with colsum correction fused
    into the PSUM->SBUF Identity-activation copy (scale=-1, bias=colsum),
    LIF2 scan, mm3 (emitted mid-way through next block's mm2 so the PE never
    waits on the DVE scan), output LIF scan, final rate.
"""

import numpy as np

# ---- problem constants (hardcoded; kernel.py must be self-contained) ----
BATCH = 256
INPUT_DIM = 1024
HIDDEN_DIM = 2048
OUTPUT_DIM = 10
T = 100
NCORES = 8
BLOC = BATCH // NCORES          # 32 batch rows per core
TB = 10                         # timesteps per block
NBLK = T // TB                  # 10 blocks
C = TB * BLOC                   # 320 columns per block
COLS = T * BLOC                 # 3200 total columns
KI = INPUT_DIM // 128           # 8 k-chunks for mm1
KH = HIDDEN_DIM // 128          # 16 k-chunks (and m-chunks) for mm2
DECAY = 0.9
THRESH = 1.0

_BUILT = None


def _build():
    """Trace + compile the Bass program once."""
    from contextlib import ExitStack

    import concourse.bacc as bacc
    import concourse.tile as tile
    from concourse import mybir
    from concourse.alu_op_type import AluOpType as op

    f32 = mybir.dt.float32
    f16 = mybir.dt.float16
    ident = mybir.ActivationFunctionType.Identity

    nc = bacc.Bacc("TRN2", target_bir_lowering=False, debug=False,
                   num_devices=NCORES)

    x_d = nc.dram_tensor("x", [INPUT_DIM, COLS], f16,
                         kind="ExternalInput").ap()
    wih_d = [nc.dram_tensor(f"wih_{s}", [INPUT_DIM, HIDDEN_DIM], f16,
                            kind="ExternalInput").ap() for s in ("hi", "lo")]
    whh_d = [nc.dram_tensor(f"whh_{s}", [HIDDEN_DIM, HIDDEN_DIM], f16,
                            kind="ExternalInput").ap() for s in ("hi", "lo")]
    who_d = [nc.dram_tensor(f"whoT_{s}", [128, KH * OUTPUT_DIM], f16,
                            kind="ExternalInput").ap() for s in ("hi", "lo")]
    cs_hh_d = nc.dram_tensor("cs_hh", [128, KH], f32, kind="ExternalInput").ap()
    cs_ho_d = nc.dram_tensor("cs_ho", [OUTPUT_DIM, 1], f32,
                             kind="ExternalInput").ap()
    out_d = nc.dram_tensor("out", [OUTPUT_DIM, BLOC], f32,
                           kind="ExternalOutput").ap()
    spill_d = nc.dram_tensor("u1spill", [KH, 128, COLS], f16,
                             kind="Internal").ap()

    with tile.TileContext(nc) as tc, ExitStack() as ctx:
        # whh hi-half preloads during phase A (fits alongside phase A pools)
        whh_pre = ctx.enter_context(tc.tile_pool(name="whhpre", bufs=1))
        whh_sb = [[], []]
        for k in range(KH):
            w = whh_pre.tile([128, HIDDEN_DIM], f16, tag=f"whh0_{k}")
            nc.sync.dma_start(w[:], whh_d[0][k * 128:(k + 1) * 128, :])
            whh_sb[0].append(w)

        # ---------------- Phase A: mm1 + LIF1 scan, spill u1 ----------------
        with tc.tile_pool(name="wih", bufs=1) as wih_pool, \
             tc.tile_pool(name="xin", bufs=2) as x_pool, \
             tc.tile_pool(name="cur1", bufs=2) as cur1_pool, \
             tc.tile_pool(name="u1b", bufs=2) as u1b_pool, \
             tc.tile_pool(name="st1", bufs=1) as st1_pool, \
             tc.tile_pool(name="psA", bufs=6, space="PSUM") as psA:

            wih_sb = [[], []]
            for h in range(2):
                for k in range(KI):
                    w = wih_pool.tile([128, HIDDEN_DIM], f16,
                                      tag=f"wih{h}_{k}")
                    nc.sync.dma_start(w[:], wih_d[h][k * 128:(k + 1) * 128, :])
                    wih_sb[h].append(w)

            y1 = st1_pool.tile([128, KH * BLOC], f32, tag="y1")
            v1 = st1_pool.tile([128, KH * BLOC], f32, tag="v1")
            nc.vector.memset(y1[:], 0.0)
            y1_3 = y1[:].rearrange("p (m b) -> p m b", m=KH)
            v1_3 = v1[:].rearrange("p (m b) -> p m b", m=KH)

            for blk in range(NBLK):
                c0 = blk * C
                xt = []
                for k in range(KI):
                    xk = x_pool.tile([128, C], f16, tag=f"x{k}")
                    nc.sync.dma_start(xk[:], x_d[k * 128:(k + 1) * 128,
                                                 c0:c0 + C])
                    xt.append(xk)
                cur1 = cur1_pool.tile([128, KH * C], f32, tag="cur1")
                u1b = u1b_pool.tile([128, KH * C], f16, tag="u1b")
                for m in range(KH):
                    ps = psA.tile([128, C], f32, tag="psA")
                    for k in range(KI):
                        for h in range(2):
                            nc.tensor.matmul(
                                ps[:],
                                wih_sb[h][k][:, m * 128:(m + 1) * 128],
                                xt[k][:], start=(k == 0 and h == 0),
                                stop=(k == KI - 1 and h == 1))
                    nc.scalar.copy(cur1[:, m * C:(m + 1) * C], ps[:])
                cur1_r = cur1[:].rearrange("p (m c) -> p m c", m=KH)
                u1b_r = u1b[:].rearrange("p (m c) -> p m c", m=KH)
                for t in range(TB):
                    sl = cur1_r[:, :, t * BLOC:(t + 1) * BLOC]
                    ub = u1b_r[:, :, t * BLOC:(t + 1) * BLOC]
                    # v = 0.9*y + cur
                    nc.vector.scalar_tensor_tensor(v1_3, y1_3, DECAY, sl,
                                                   op.mult, op.add)
                    # u = (v < 1), fp16 {0,1} for the next matmul
                    nc.vector.tensor_scalar(ub, v1_3, THRESH, None, op.is_lt)
                    # y = (v<1)*v
                    nc.vector.scalar_tensor_tensor(y1_3, v1_3, THRESH, v1_3,
                                                   op.is_lt, op.mult)
                for m in range(KH):
                    nc.sync.dma_start(spill_d[m, :, c0:c0 + C],
                                      u1b[:, m * C:(m + 1) * C])

        # ---------------- Phase B: mm2 + LIF2 + mm3 + output scan -----------
        with tc.tile_pool(name="whh", bufs=1) as whh_pool, \
             tc.tile_pool(name="u1", bufs=2) as u1_pool, \
             tc.tile_pool(name="cur2", bufs=2) as cur2_pool, \
             tc.tile_pool(name="u2bp", bufs=1) as u2b_pool, \
             tc.tile_pool(name="smallB", bufs=1) as sm_pool, \
             tc.tile_pool(name="cur3", bufs=1) as cur3_pool, \
             tc.tile_pool(name="psB", bufs=6, space="PSUM") as psB, \
             tc.tile_pool(name="ps3", bufs=2, space="PSUM") as ps3_pool:

            for k in range(KH):
                w = whh_pool.tile([128, HIDDEN_DIM], f16, tag=f"whh1_{k}")
                nc.sync.dma_start(w[:], whh_d[1][k * 128:(k + 1) * 128, :])
                whh_sb[1].append(w)
            who_sb = []
            for h in range(2):
                w = sm_pool.tile([128, KH * OUTPUT_DIM], f16, tag=f"who{h}")
                nc.sync.dma_start(w[:], who_d[h][:, :])
                who_sb.append(w)
            cs_hh = sm_pool.tile([128, KH], f32, tag="cshh")
            nc.sync.dma_start(cs_hh[:], cs_hh_d[:, :])
            cs_ho = sm_pool.tile([OUTPUT_DIM, 1], f32, tag="csho")
            nc.sync.dma_start(cs_ho[:], cs_ho_d[:, :])

            y2 = sm_pool.tile([128, KH * BLOC], f32, tag="y2")
            v2 = sm_pool.tile([128, KH * BLOC], f32, tag="v2")
            yo = sm_pool.tile([OUTPUT_DIM, BLOC], f32, tag="yo")
            vo = sm_pool.tile([OUTPUT_DIM, BLOC], f32, tag="vo")
            acc0 = sm_pool.tile([OUTPUT_DIM, BLOC], f32, tag="acc0")
            acc1 = sm_pool.tile([OUTPUT_DIM, BLOC], f32, tag="acc1")
            acc = [acc0, acc1]
            out_sb = sm_pool.tile([OUTPUT_DIM, BLOC], f32, tag="rate")
            nc.vector.memset(y2[:], 0.0)
            nc.vector.memset(yo[:], 0.0)
            nc.vector.memset(acc[0][:], 0.0)
            y2_3 = y2[:].rearrange("p (m b) -> p m b", m=KH)
            v2_3 = v2[:].rearrange("p (m b) -> p m b", m=KH)

            u2b = u2b_pool.tile([128, KH * C], f16, tag="u2b")
            u2b_r = u2b[:].rearrange("p (m c) -> p m c", m=KH)

            def emit_mm3(blk):
                """mm3 + output-layer scan for a finished block."""
                ps3 = ps3_pool.tile([OUTPUT_DIM, C], f32, tag="ps3")
                for k in range(KH):
                    for h in range(2):
                        nc.tensor.matmul(
                            ps3[:],
                            who_sb[h][:, k * OUTPUT_DIM:(k + 1) * OUTPUT_DIM],
                            u2b[:, k * C:(k + 1) * C],
                            start=(k == 0 and h == 0),
                            stop=(k == KH - 1 and h == 1))
                cur3 = cur3_pool.tile([OUTPUT_DIM, C], f32, tag="cur3")
                # cur3 = colsum_ho - u2@W_ho  (true output current)
                nc.scalar.activation(cur3[:], ps3[:], ident,
                                     bias=cs_ho[:, 0:1], scale=-1.0)
                for t in range(TB):
                    g = blk * TB + t
                    sl = cur3[:, t * BLOC:(t + 1) * BLOC]
                    nc.vector.scalar_tensor_tensor(vo[:], yo[:], DECAY, sl,
                                                   op.mult, op.add)
                    nc.vector.scalar_tensor_tensor(acc[(g + 1) % 2][:], vo[:],
                                                   THRESH, acc[g % 2][:],
                                                   op.is_lt, op.add)
                    nc.vector.scalar_tensor_tensor(yo[:], vo[:], THRESH, vo[:],
                                                   op.is_lt, op.mult)

            prev = None
            for blk in range(NBLK):
                c0 = blk * C
                u1 = u1_pool.tile([128, KH * C], f16, tag="u1")
                for m in range(KH):
                    nc.sync.dma_start(u1[:, m * C:(m + 1) * C],
                                      spill_d[m, :, c0:c0 + C])
                cur2 = cur2_pool.tile([128, KH * C], f32, tag="cur2")
                for m2 in range(KH):
                    if m2 == 8 and prev is not None:
                        emit_mm3(prev)
                        prev = None
                    ps = psB.tile([128, C], f32, tag="psB")
                    for h in range(2):
                        for k in range(KH):
                            nc.tensor.matmul(
                                ps[:],
                                whh_sb[h][k][:, m2 * 128:(m2 + 1) * 128],
                                u1[:, k * C:(k + 1) * C],
                                start=(h == 0 and k == 0),
                                stop=(h == 1 and k == KH - 1))
                    # cur2 = colsum_hh - u1@W_hh  (true layer-2 current)
                    nc.scalar.activation(cur2[:, m2 * C:(m2 + 1) * C], ps[:],
                                         ident, bias=cs_hh[:, m2:m2 + 1],
                                         scale=-1.0)
                cur2_r = cur2[:].rearrange("p (m c) -> p m c", m=KH)
                for t in range(TB):
                    sl = cur2_r[:, :, t * BLOC:(t + 1) * BLOC]
                    ub = u2b_r[:, :, t * BLOC:(t + 1) * BLOC]
                    nc.vector.scalar_tensor_tensor(v2_3, y2_3, DECAY, sl,
                                                   op.mult, op.add)
                    nc.vector.tensor_scalar(ub, v2_3, THRESH, None, op.is_lt)
                    nc.vector.scalar_tensor_tensor(y2_3, v2_3, THRESH, v2_3,
                                                   op.is_lt, op.mult)
                prev = blk
            emit_mm3(prev)

            # rate = 1 - acc/T   (acc holds sum of u_out; s = 1-u)
            nc.vector.tensor_scalar(out_sb[:], acc[T % 2][:], -1.0 / T, 1.0,
                                    op.mult, op.add)
            nc.sync.dma_start(out_d[:, :], out_sb[:])

    nc.compile()
    return nc


def _split_f16(w):
    hi = w.astype(np.float16)
    lo = (w - hi.astype(np.float32)).astype(np.float16)
    return np.ascontiguousarray(hi), np.ascontiguousarray(lo)


def kernel(input_bins, W_ih, W_hh, W_ho):
    global _BUILT
    if _BUILT is None:
        _BUILT = _build()
    nc = _BUILT
    input_bins = np.ascontiguousarray(input_bins, dtype=np.float32)
    W_ih = np.ascontiguousarray(W_ih, dtype=np.float32)
    W_hh2 = np.ascontiguousarray(np.asarray(W_hh)[0], dtype=np.float32)
    W_ho = np.ascontiguousarray(W_ho, dtype=np.float32)

    wih_hi, wih_lo = _split_f16(W_ih)
    whh_hi, whh_lo = _split_f16(W_hh2)
    # lhsT layout for mm3: [k-chunk, 128, 10] -> [128, 16*10]
    whoT = np.ascontiguousarray(
        W_ho.reshape(KH, 128, OUTPUT_DIM).transpose(1, 0, 2).reshape(
            128, KH * OUTPUT_DIM))
    whoT_hi, whoT_lo = _split_f16(whoT)
    cs_hh = np.ascontiguousarray(
        W_hh2.sum(axis=0, dtype=np.float32).reshape(KH, 128).T)
    cs_ho = W_ho.sum(axis=0, dtype=np.float32).reshape(OUTPUT_DIM, 1)

    in_maps = []
    for c in range(NCORES):
        xb = input_bins[c * BLOC:(c + 1) * BLOC]        # [32, 1024, 100]
        # -> [input_dim, t, b] -> [1024, 3200] (t-major columns)
        xc = np.ascontiguousarray(
            xb.transpose(1, 2, 0).reshape(INPUT_DIM, COLS)
            .astype(np.float16))
        in_maps.append({
            "x": xc, "wih_hi": wih_hi, "wih_lo": wih_lo,
            "whh_hi": whh_hi, "whh_lo": whh_lo,
            "whoT_hi": whoT_hi, "whoT_lo": whoT_lo,
            "cs_hh": cs_hh, "cs_ho": cs_ho,
        })

    from concourse.bass_utils import run_bass_kernel_spmd
    res = run_bass_kernel_spmd(nc, in_maps, core_ids=list(range(NCORES)))

    out = np.empty((BATCH, OUTPUT_DIM), dtype=np.float32)
    for c in range(NCORES):
        out[c * BLOC:(c + 1) * BLOC] = res.results[c]["out"].T
    return out



# revision 7
# speedup vs baseline: 1.2935x; 1.2935x over previous
"""Trainium2 Bass kernel for a 2-hidden-layer LIF spiking network.

Math (per timestep t, per layer):
    v = 0.9*y + cur ;  spike s = (v >= 1) ;  y = v*(1-s) = v*u  with u = (v < 1)
Layer currents:
    cur1 = x_t @ W_ih            (x binary, precomputable for ALL t)
    cur2 = s1 @ W_hh = colsum(W_hh) - u1 @ W_hh
    cur3 = s2 @ W_ho = colsum(W_ho) - u2 @ W_ho
Output: rate = mean_t s_out = 1 - sum_t(u_out)/T

Key restructurings:
  * Layer 1's recurrence does not depend on layer 2, so all three matmuls are
    batched over the full (T*B) column space; only the cheap elementwise LIF
    scans are sequential in t.
  * Weights are quantized to 24-bit fixed point (step 2^-K) and decomposed
    into ND=6 exact signed base-16 digit planes, each stored in fp8 e5m2
    (digits in [-8,7] and power-of-2 scales are exact in e5m2). The moving
    operands (x and the spike complements u) carry the value 2^-14, exactly
    representable as the e5m2 minimum normal. Pairs of digit planes feed
    fp8 DoubleRow matmuls (2 stationary planes per instruction at 0.5
    cycles/row), so full 24-bit weight precision streams at 1.5 cycles/row
    vs 2.0 for an fp16 hi/lo pair -- with every product exact in fp32 PSUM.
  * The moving AP broadcasts the same spike tile across the DoubleRow pair
    (middle dim stride 0), so spikes are stored once.

Sharding: data-parallel over batch (256/8 = 32 rows per core), weights
replicated, no cross-core communication.

Per-core schedule:
  Phase A (W_ih digit planes resident, 12.6MB): mm1 over blocks of 10 steps,
    LIF1 scan fused per block, spill u1 (e5m2 {0,2^-14}) to DRAM scratch.
  Phase B (W_hh digit planes streamed from DRAM per 128-col output chunk,
    double-buffered): superblocks of 20 steps; mm2 -> cur2 with colsum
    correction fused into the PSUM->SBUF Identity-activation copy
    (scale=-1, bias=colsum), LIF2 scan, mm3 (emitted mid-way through the
    next superblock's mm2 so the PE never waits on the DVE scan), output
    LIF scan, final rate.
"""

import numpy as np

# ---- problem constants (hardcoded; kernel.py must be self-contained) ----
BATCH = 256
INPUT_DIM = 1024
HIDDEN_DIM = 2048
OUTPUT_DIM = 10
T = 100
NCORES = 8
BLOC = BATCH // NCORES          # 32 batch rows per core
TB = 10                         # timesteps per phase-A block
NBLK = T // TB                  # 10 blocks
C = TB * BLOC                   # 320 columns per block
TS = 20                         # timesteps per phase-B superblock
NSUP = T // TS                  # 5 superblocks
SC = TS * BLOC                  # 640 columns per superblock
COLS = T * BLOC                 # 3200 total columns
KI = INPUT_DIM // 128           # 8 k-chunks for mm1
KH = HIDDEN_DIM // 128          # 16 k-chunks (and m-chunks) for mm2
DECAY = 0.9
THRESH = 1.0
TH_NUDGE = 0.0                  # tie-break re-roll knob (harmless ~1e-6 scale)

ND = 6                          # digit planes (24-bit fixed point)
KBITS = 23                      # weight step 2^-KBITS (max digit range 7829367)
MOV = 2.0 ** -14                # moving-operand value (e5m2 min normal)
DMAX = 7 * (16 ** ND - 1) // 15

_BUILT = None


def _build():
    """Trace + compile the Bass program once."""
    from contextlib import ExitStack

    import concourse.bacc as bacc
    import concourse.tile as tile
    from concourse import mybir
    from concourse.alu_op_type import AluOpType as op

    f32 = mybir.dt.float32
    e5 = mybir.dt.float8e5
    DR = mybir.MatmulPerfMode.DoubleRow
    ident = mybir.ActivationFunctionType.Identity
    TH = THRESH + TH_NUDGE

    nc = bacc.Bacc("TRN2", target_bir_lowering=False, debug=False,
                   num_devices=NCORES)

    # x values {0, 2^-14}: [input_dim, t*b] t-major columns
    x_d = nc.dram_tensor("x", [INPUT_DIM, COLS], e5,
                         kind="ExternalInput").ap()
    # digit planes: wih [kt*128, dig*HID]; whh [(m2*KH+kt)*128, dig*128]
    wih_d = nc.dram_tensor("wihd", [INPUT_DIM, ND * HIDDEN_DIM], e5,
                           kind="ExternalInput").ap()
    whh_d = nc.dram_tensor("whhd", [KH * HIDDEN_DIM, ND * 128], e5,
                           kind="ExternalInput").ap()
    # who planes padded to 16 cols: [(kt)*128, dig*16]
    who_d = nc.dram_tensor("whod", [KH * 128, ND * 16], e5,
                           kind="ExternalInput").ap()
    cs_hh_d = nc.dram_tensor("cs_hh", [128, KH], f32, kind="ExternalInput").ap()
    cs_ho_d = nc.dram_tensor("cs_ho", [OUTPUT_DIM, 1], f32,
                             kind="ExternalInput").ap()
    out_d = nc.dram_tensor("out", [OUTPUT_DIM, BLOC], f32,
                           kind="ExternalOutput").ap()
    spill_d = nc.dram_tensor("u1spill", [KH, 128, COLS], e5,
                             kind="Internal").ap()

    with tile.TileContext(nc) as tc, ExitStack() as ctx:
        # ---------------- Phase A: mm1 + LIF1 scan, spill u1 ----------------
        with tc.tile_pool(name="wih", bufs=1) as wih_pool, \
             tc.tile_pool(name="xin", bufs=2) as x_pool, \
             tc.tile_pool(name="cur1", bufs=2) as cur1_pool, \
             tc.tile_pool(name="u1b", bufs=2) as u1b_pool, \
             tc.tile_pool(name="st1", bufs=1) as st1_pool, \
             tc.tile_pool(name="psA", bufs=6, space="PSUM") as psA:

            wih_sb = []
            for k in range(KI):
                w = wih_pool.tile([128, ND * HIDDEN_DIM], e5, tag=f"wih_{k}")
                nc.sync.dma_start(w[:], wih_d[k * 128:(k + 1) * 128, :])
                wih_sb.append(w)

            y1 = st1_pool.tile([128, KH * BLOC], f32, tag="y1")
            v1 = st1_pool.tile([128, KH * BLOC], f32, tag="v1")
            nc.vector.memset(y1[:], 0.0)
            y1_3 = y1[:].rearrange("p (m b) -> p m b", m=KH)
            v1_3 = v1[:].rearrange("p (m b) -> p m b", m=KH)

            for blk in range(NBLK):
                c0 = blk * C
                xt = []
                for k in range(KI):
                    xk = x_pool.tile([128, C], e5, tag=f"x{k}")
                    nc.sync.dma_start(xk[:], x_d[k * 128:(k + 1) * 128,
                                                 c0:c0 + C])
                    xt.append(xk)
                cur1 = cur1_pool.tile([128, KH * C], f32, tag="cur1")
                u1b = u1b_pool.tile([128, KH * C], e5, tag="u1b")
                for m in range(KH):
                    ps = psA.tile([128, C], f32, tag="psA")
                    for k in range(KI):
                        wk3 = wih_sb[k][:].rearrange("p (i m) -> p i m", i=ND)
                        xb = xt[k][:].unsqueeze(1).broadcast_to([128, 2, C])
                        for j in range(ND // 2):
                            nc.tensor.matmul(
                                ps[:],
                                wk3[:, 2 * j:2 * j + 2,
                                    m * 128:(m + 1) * 128],
                                xb,
                                start=(k == 0 and j == 0),
                                stop=(k == KI - 1 and j == ND // 2 - 1),
                                perf_mode=DR)
                    nc.scalar.copy(cur1[:, m * C:(m + 1) * C], ps[:])
                cur1_r = cur1[:].rearrange("p (m c) -> p m c", m=KH)
                u1b_r = u1b[:].rearrange("p (m c) -> p m c", m=KH)
                for t in range(TB):
                    sl = cur1_r[:, :, t * BLOC:(t + 1) * BLOC]
                    ub = u1b_r[:, :, t * BLOC:(t + 1) * BLOC]
                    # v = 0.9*y + cur
                    nc.vector.scalar_tensor_tensor(v1_3, y1_3, DECAY, sl,
                                                   op.mult, op.add)
                    # u = (v < 1) * 2^-14, e5m2 for the DoubleRow matmul
                    nc.vector.tensor_scalar(ub, v1_3, TH, MOV,
                                            op.is_lt, op.mult)
                    # y = (v<1)*v
                    nc.vector.scalar_tensor_tensor(y1_3, v1_3, TH, v1_3,
                                                   op.is_lt, op.mult)
                for m in range(KH):
                    nc.sync.dma_start(spill_d[m, :, c0:c0 + C],
                                      u1b[:, m * C:(m + 1) * C])

        # ---------------- Phase B: mm2 + LIF2 + mm3 + output scan -----------
        with tc.tile_pool(name="u1", bufs=1) as u1_pool, \
             tc.tile_pool(name="wst", bufs=2) as wst_pool, \
             tc.tile_pool(name="cur2", bufs=2) as cur2_pool, \
             tc.tile_pool(name="u2p", bufs=2) as u2_pool, \
             tc.tile_pool(name="smallB", bufs=1) as sm_pool, \
             tc.tile_pool(name="cur3", bufs=2) as cur3_pool, \
             tc.tile_pool(name="psB", bufs=6, space="PSUM") as psB, \
             tc.tile_pool(name="ps3", bufs=2, space="PSUM") as ps3_pool:

            who_sb = sm_pool.tile([128, KH * ND * 16], e5, tag="who")
            nc.sync.dma_start(
                who_sb[:].rearrange("p (k f) -> p k f", k=KH),
                who_d[:, :].rearrange("(k p) f -> p k f", p=128))
            who4 = who_sb[:].rearrange("p (k i m) -> p k i m", k=KH, i=ND)
            cs_hh = sm_pool.tile([128, KH], f32, tag="cshh")
            nc.sync.dma_start(cs_hh[:], cs_hh_d[:, :])
            cs_ho = sm_pool.tile([OUTPUT_DIM, 1], f32, tag="csho")
            nc.sync.dma_start(cs_ho[:], cs_ho_d[:, :])

            y2 = sm_pool.tile([128, KH * BLOC], f32, tag="y2")
            v2 = sm_pool.tile([128, KH * BLOC], f32, tag="v2")
            yo = sm_pool.tile([OUTPUT_DIM, BLOC], f32, tag="yo")
            vo = sm_pool.tile([OUTPUT_DIM, BLOC], f32, tag="vo")
            acc0 = sm_pool.tile([OUTPUT_DIM, BLOC], f32, tag="acc0")
            acc1 = sm_pool.tile([OUTPUT_DIM, BLOC], f32, tag="acc1")
            acc = [acc0, acc1]
            out_sb = sm_pool.tile([OUTPUT_DIM, BLOC], f32, tag="rate")
            nc.vector.memset(y2[:], 0.0)
            nc.vector.memset(yo[:], 0.0)
            nc.vector.memset(acc[0][:], 0.0)
            y2_3 = y2[:].rearrange("p (m b) -> p m b", m=KH)
            v2_3 = v2[:].rearrange("p (m b) -> p m b", m=KH)

            # u1 resident for the whole phase: [p, kt, col]
            u1 = u1_pool.tile([128, KH * COLS], e5, tag="u1")
            u1_3 = u1[:].rearrange("p (k c) -> p k c", k=KH)

            def emit_mm3(sup, u2_3):
                """mm3 + output-layer scan for a finished superblock."""
                cur3 = cur3_pool.tile([OUTPUT_DIM, SC], f32, tag="cur3")
                for nh in range(2):
                    ps3 = ps3_pool.tile([OUTPUT_DIM, C], f32, tag="ps3")
                    for k in range(KH):
                        ub = u2_3[:, k, nh * C:(nh + 1) * C] \
                            .unsqueeze(1).broadcast_to([128, 2, C])
                        for j in range(ND // 2):
                            nc.tensor.matmul(
                                ps3[:],
                                who4[:, k, 2 * j:2 * j + 2, 0:OUTPUT_DIM],
                                ub,
                                start=(k == 0 and j == 0),
                                stop=(k == KH - 1 and j == ND // 2 - 1),
                                perf_mode=DR)
                    # cur3 = colsum_ho - u2@W_ho  (true output current)
                    nc.scalar.activation(cur3[:, nh * C:(nh + 1) * C],
                                         ps3[:], ident,
                                         bias=cs_ho[:, 0:1], scale=-1.0)
                for t in range(TS):
                    g = sup * TS + t
                    sl = cur3[:, t * BLOC:(t + 1) * BLOC]
                    nc.vector.scalar_tensor_tensor(vo[:], yo[:], DECAY, sl,
                                                   op.mult, op.add)
                    nc.vector.scalar_tensor_tensor(acc[(g + 1) % 2][:], vo[:],
                                                   TH, acc[g % 2][:],
                                                   op.is_lt, op.add)
                    nc.vector.scalar_tensor_tensor(yo[:], vo[:], TH, vo[:],
                                                   op.is_lt, op.mult)

            prev = None
            for sup in range(NSUP):
                c0 = sup * SC
                for k in range(KH):
                    nc.sync.dma_start(u1_3[:, k, c0:c0 + SC],
                                      spill_d[k, :, c0:c0 + SC])
                cur2 = cur2_pool.tile([128, KH * SC], f32, tag="cur2")
                u2 = u2_pool.tile([128, KH * SC], e5, tag="u2")
                u2_3 = u2[:].rearrange("p (m c) -> p m c", m=KH)
                for m2 in range(KH):
                    if m2 == 8 and prev is not None:
                        emit_mm3(*prev)
                        prev = None
                    wst = wst_pool.tile([128, KH * ND * 128], e5, tag="wst")
                    nc.sync.dma_start(
                        wst[:].rearrange("p (k f) -> p k f", k=KH),
                        whh_d[m2 * HIDDEN_DIM:(m2 + 1) * HIDDEN_DIM, :]
                        .rearrange("(k p) f -> p k f", p=128))
                    wst4 = wst[:].rearrange("p (k i m) -> p k i m",
                                            k=KH, i=ND)
                    for nh in range(2):
                        ps = psB.tile([128, C], f32, tag="psB")
                        for k in range(KH):
                            ub = u1_3[:, k, c0 + nh * C:c0 + (nh + 1) * C] \
                                .unsqueeze(1).broadcast_to([128, 2, C])
                            for j in range(ND // 2):
                                nc.tensor.matmul(
                                    ps[:],
                                    wst4[:, k, 2 * j:2 * j + 2, :],
                                    ub,
                                    start=(k == 0 and j == 0),
                                    stop=(k == KH - 1 and j == ND // 2 - 1),
                                    perf_mode=DR)
                        # cur2 = colsum_hh - u1@W_hh  (true layer-2 current)
                        nc.scalar.activation(
                            cur2[:, m2 * SC + nh * C:m2 * SC + (nh + 1) * C],
                            ps[:], ident, bias=cs_hh[:, m2:m2 + 1],
                            scale=-1.0)
                cur2_r = cur2[:].rearrange("p (m c) -> p m c", m=KH)
                for t in range(TS):
                    sl = cur2_r[:, :, t * BLOC:(t + 1) * BLOC]
                    ub = u2_3[:, :, t * BLOC:(t + 1) * BLOC]
                    nc.vector.scalar_tensor_tensor(v2_3, y2_3, DECAY, sl,
                                                   op.mult, op.add)
                    nc.vector.tensor_scalar(ub, v2_3, TH, MOV,
                                            op.is_lt, op.mult)
                    nc.vector.scalar_tensor_tensor(y2_3, v2_3, TH, v2_3,
                                                   op.is_lt, op.mult)
                prev = (sup, u2_3)
            emit_mm3(*prev)

            # rate = 1 - acc/T   (acc holds sum of u_out; s = 1-u)
            nc.vector.tensor_scalar(out_sb[:], acc[T % 2][:], -1.0 / T, 1.0,
                                    op.mult, op.add)
            nc.sync.dma_start(out_d[:, :], out_sb[:])

    nc.compile()
    return nc


def _digit_planes(w):
    """Decompose fp32 weights into ND exact e5m2 digit planes.

    w ~= Wfix * 2^-KBITS with Wfix = sum_i d_i 16^i, d_i in [-8,7].
    Plane i holds d_i * 2^(4i - KBITS + 14); the moving operand carries
    2^-14, so plane_i * moving accumulates to exactly Wfix * 2^-KBITS.
    Returns (planes [ND, *w.shape] e5m2-exact fp32, effective weights fp32).
    """
    wfix = np.round(w.astype(np.float64) * (1 << KBITS)).astype(np.int64)
    assert np.abs(wfix).max() <= DMAX, "weights exceed digit range"
    planes = np.zeros((ND,) + w.shape, np.float32)
    rem = wfix.copy()
    for i in range(ND):
        d = ((rem + 8) % 16) - 8
        rem = (rem - d) >> 4
        planes[i] = d * np.float32(2.0 ** (4 * i - KBITS + 14))
    assert np.all(rem == 0)
    weff = (wfix * (2.0 ** -KBITS)).astype(np.float32)
    return planes, weff


def kernel(input_bins, W_ih, W_hh, W_ho):
    global _BUILT
    if _BUILT is None:
        _BUILT = _build()
    nc = _BUILT
    import ml_dtypes
    e5np = ml_dtypes.float8_e5m2

    input_bins = np.ascontiguousarray(input_bins, dtype=np.float32)
    W_ih = np.ascontiguousarray(W_ih, dtype=np.float32)
    W_hh2 = np.ascontiguousarray(np.asarray(W_hh)[0], dtype=np.float32)
    W_ho = np.ascontiguousarray(W_ho, dtype=np.float32)

    pih, wih_eff = _digit_planes(W_ih)       # [ND, 1024, 2048]
    phh, whh_eff = _digit_planes(W_hh2)      # [ND, 2048, 2048]
    pho, who_eff = _digit_planes(W_ho)       # [ND, 2048, 10]

    # wih planes -> [kt*128, dig*HID]
    wihd = np.ascontiguousarray(
        pih.transpose(1, 0, 2).reshape(INPUT_DIM, ND * HIDDEN_DIM)
    ).astype(e5np)
    # whh planes -> [(m2*KH + kt)*128, dig*128]
    whhd = np.ascontiguousarray(
        phh.reshape(ND, KH, 128, KH, 128)      # [dig, kt, p, m2, mc]
        .transpose(3, 1, 2, 0, 4)              # [m2, kt, p, dig, mc]
        .reshape(KH * HIDDEN_DIM, ND * 128)
    ).astype(e5np)
    # who planes padded to 16 output cols: [128, dig, 16] per kt stacked in p?
    # layout [p, dig*16] with the KH k-chunks... who is [2048, 10]: k-chunks
    # on partitions per chunk; store as [128, ND*16] per k-chunk stacked along
    # free? lhsT slice is per k-chunk [128, 2, 10] -> need per-kt tiles.
    whod = np.zeros((KH, 128, ND, 16), np.float32)
    whod[:, :, :, :OUTPUT_DIM] = pho.reshape(ND, KH, 128, OUTPUT_DIM) \
        .transpose(1, 2, 0, 3)
    whod8 = np.ascontiguousarray(whod.reshape(KH * 128, ND * 16)).astype(e5np)

    cs_hh = np.ascontiguousarray(
        whh_eff.sum(axis=0, dtype=np.float64).astype(np.float32)
        .reshape(KH, 128).T)
    cs_ho = who_eff.sum(axis=0, dtype=np.float64).astype(np.float32) \
        .reshape(OUTPUT_DIM, 1)

    in_maps = []
    for c in range(NCORES):
        xb = input_bins[c * BLOC:(c + 1) * BLOC]        # [32, 1024, 100]
        xc = np.ascontiguousarray(
            xb.transpose(1, 2, 0).reshape(INPUT_DIM, COLS) * np.float32(MOV)
        ).astype(e5np)
        in_maps.append({
            "x": xc, "wihd": wihd, "whhd": whhd, "whod": whod8,
            "cs_hh": cs_hh, "cs_ho": cs_ho,
        })

    from concourse.bass_utils import run_bass_kernel_spmd
    res = run_bass_kernel_spmd(nc, in_maps, core_ids=list(range(NCORES)))

    out = np.empty((BATCH, OUTPUT_DIM), dtype=np.float32)
    for c in range(NCORES):
        out[c * BLOC:(c + 1) * BLOC] = res.results[c]["out"].T
    return out
